# revision 33
# baseline (speedup 1.0000x reference)
"""Trainium2 Bass kernel for masked-softmax attention (sparse_attention).

Computes, for full inputs
    x           [H=4, N=4096, D=256] f32
    adj         [N, N] int32 (0/1)
    att_pattern [H, N, N] f32
the reference
    score = leaky_relu(att_pattern, 0.2)
    score = where(adj > 0, score, -9e15)
    ratio = softmax(score, axis=-1)
    out   = einsum('hnm,hmd->hnd', ratio, x)

Sharding: head-parallel (per the sharding hint) — core c handles head c//2,
row half c%2 (2048 rows), so each core needs only its own head's x (2.1MB)
instead of a replicated 8.4MB slab.

Host-side marshalling: adj and the elementwise leaky_relu are folded into
the score tensor on the host (s = where(adj, leaky_relu(att), -60) in f16;
exp(-60) -> 0 exactly), so the mask costs zero HBM traffic and the
score-prep costs zero DVE work on device. Scores ship f16 PRE-TRANSPOSED
into the [m-on-partitions, rows-free] layout the PE matmul wants for lhsT.
x ships f16 with a ones-column appended (the ones column makes the
accumulating matmul produce row-sums for free).

The device computes the softmax-attention proper, per 128-row tile
(at = masked score^T tile, f16):
    e  = exp(at)                  (ACT; scores <= ~5.7 so e <= ~300, no
                                   max-subtraction needed; ~59us of exp)
    psum[rows, 0:256] += e.T @ x_chunk ; psum[rows, 256] += rowsum(e)
                                  (PE; 32 accumulating 128x128 @ 128x257
                                   matmuls per tile, ~62us warm — the
                                   pacing engine)
    out_rows = psum[:, :256] * (1 / psum[:, 256])   (DVE normalize, lagged
                                   two tiles so it never idles on PSUM)
fp16 data path, fp32 PSUM accumulation, f16 output (host casts f32).

DMA (~20MB at ~330GB/s paces the ramp): att streams as 1MB singles — the
first two tiles as 0.5MB halves — with the x slab interleaved as four
0.53MB quarters so the first matmuls start at ~13us without displacing the
att deliveries the exp stream needs; an 8-deep att ring keeps DMA posts
from ever WAR-waiting on exp completions. Output leaves in three stores
(8/7/1 tiles) so the drain chain ends on a 0.13MB transfer. Tiny warm-up
matmuls tied to the first att arrivals keep the HAM clock gate from
re-throttling PE across ramp gaps.
"""

import numpy as np

import concourse.bass as bass
import concourse.mybir as mybir
import concourse.tile as tile
from concourse import bacc
from concourse.bass_utils import run_bass_kernel_spmd

H, N, D = 4, 4096, 256
NCORES = 8
R = N // 2               # rows per core = 2048 (half a head)
RBLKS = R // 128         # 128-row blocks per core = 16
KC = N // 128            # contraction chunks = 32
DP1 = D + 1              # matmul rhs width (ones column appended)
HN = N // 2              # half a tile's free dim (= chunks 0..15)
MASKVAL = np.float16(-60.0)   # exp(-60) -> 0 exactly in f16

f32 = mybir.dt.float32
f16 = mybir.dt.float16
AF = mybir.ActivationFunctionType
OP = mybir.AluOpType

def _emit(ctx, tc: tile.TileContext, attm: bass.AP, xb16: bass.AP,
          out: bass.AP):
    nc = tc.nc

    xpool = ctx.enter_context(tc.tile_pool(name="xpool", bufs=1))
    # 8-deep singles ring: the att stream must never wait on exp completions
    # (a 4-deep ring made every DMA post WAR-wait on an exp 4 tiles back).
    atsing = ctx.enter_context(tc.tile_pool(name="atsing", bufs=10))
    esing = ctx.enter_context(tc.tile_pool(name="esing", bufs=5))
    opool = ctx.enter_context(tc.tile_pool(name="opool", bufs=3))
    rpool = ctx.enter_context(tc.tile_pool(name="rpool", bufs=2))
    psum_o = ctx.enter_context(tc.tile_pool(name="psum_o", bufs=6, space="PSUM"))

    # x slab for this core's head, loaded once as four 0.53MB quarters
    # interleaved between the first att tiles: tile 0's matmuls start after
    # the first quarter, and the 3.7us dribble keeps PE busy often enough
    # that the HAM clock-gate never re-throttles it during the ramp.
    xs = xpool.tile([128, KC, DP1], f16, tag="xs", name="xs")
    KQ = KC // 4

    def post_xs_quarter(q):
        nc.sync.dma_start(
            xs[:, q * KQ:(q + 1) * KQ, :],
            xb16[:, q * KQ * DP1:(q + 1) * KQ * DP1]
            .rearrange("p (k d) -> p k d", k=KQ))

    at_of = {}               # tile index -> its [128, N] AP

    def post_single(j, parts=1):
        at = atsing.tile([128, N], f16, tag="ats", name=f"at{j}")
        at_of[j] = at
        step = N // parts
        for p in range(parts):
            nc.sync.dma_start(at[:, p * step:(p + 1) * step],
                              attm[j][:, p * step:(p + 1) * step])

    # att delivery: tiles 0,1 in 0.5MB halves (earliest possible exp starts)
    # with xs quarters interleaved, then 1MB singles throughout — pair-sized
    # transfers made the exp stream wait on 2MB-atomic deliveries.
    post_single(0, parts=2)
    post_xs_quarter(0)
    post_single(1, parts=2)
    post_xs_quarter(1)
    post_single(2)
    post_xs_quarter(2)
    post_single(3)
    post_xs_quarter(3)

    e_of = {}
    po_of = {}
    obufs = {}

    def mm(j, ks, ke):
        """accumulate psum[j] over contraction chunks [ks, ke)."""
        if j not in po_of:
            po_of[j] = psum_o.tile([128, DP1], f32, tag="po", name=f"po{j}")
        po = po_of[j]
        e = e_of[j]
        for kk in range(ks, ke):
            nc.tensor.matmul(
                po,
                lhsT=e[:, kk * 128:(kk + 1) * 128],
                rhs=xs[:, kk, :],
                start=(kk == 0),
                stop=(kk == KC - 1),
            )

    # PE clock-gate warm-up: the HAM re-throttles PE to 1.2GHz after ~5.2us
    # idle, so 65ns matmuls tied to the first att arrivals bridge the ramp
    # gaps between PE's preamble and the real matmul stream. The scratch
    # PSUM is never read.
    wpool = ctx.enter_context(tc.tile_pool(name="wpool", bufs=1))
    wscr = wpool.tile([128, 8], f16, tag="wscr", name="wscr")
    nc.gpsimd.memset(wscr, 0.0)
    psum_w = ctx.enter_context(tc.tile_pool(name="psum_w", bufs=1, space="PSUM"))
    warm_psum = psum_w.tile([128, 8], f32, tag="warm", name="warm_psum")

    def warm_mm(src):
        nc.tensor.matmul(warm_psum, lhsT=src[:, :128], rhs=wscr,
                         start=True, stop=True)

    # output store groups: tiles 0-7, 8-14, then 15 alone so the final
    # store in the drain chain is only 0.13MB
    OGRP = {j: (0, j, 8) for j in range(8)}
    OGRP.update({j: (1, j - 8, 7) for j in range(8, 15)})
    OGRP[15] = (2, 0, 1)
    OBASE = {0: 0, 1: 8, 2: 15}

    def norm(j):
        po = po_of[j]
        rec = rpool.tile([128, 1], f32, tag="rec", name=f"rec{j}")
        nc.vector.reciprocal(rec, po[:, D:DP1])
        g, slot, gsize = OGRP[j]
        if slot == 0:
            obufs[g] = opool.tile([128, 8, D], f16, tag="o", name=f"o{g}")
        nc.vector.tensor_scalar_mul(obufs[g][:, slot, :], po[:, :D], rec)
        if slot == gsize - 1:
            nc.sync.dma_start(out[:, OBASE[g]:OBASE[g] + gsize, :],
                              obufs[g][:, :gsize, :])

    def exp_single(j):
        e = esing.tile([128, N], f16, tag="es", name=f"e{j}")
        nc.scalar.activation(e, at_of[j], AF.Exp)
        e_of[j] = e

    # --- tiles 0,1: exp in halves, tile-0 matmuls split per xs quarter ----
    warm_mm(at_of[0][:, :HN])
    e0 = esing.tile([128, N], f16, tag="es", name="e0")
    nc.scalar.activation(e0[:, :HN], at_of[0][:, :HN], AF.Exp)
    e_of[0] = e0
    warm_mm(at_of[0][:, HN:])
    mm(0, 0, KQ)
    mm(0, KQ, 2 * KQ)
    nc.scalar.activation(e0[:, HN:], at_of[0][:, HN:], AF.Exp)
    mm(0, 2 * KQ, 3 * KQ)
    mm(0, 3 * KQ, KC)
    e1 = esing.tile([128, N], f16, tag="es", name="e1")
    nc.scalar.activation(e1[:, :HN], at_of[1][:, :HN], AF.Exp)
    e_of[1] = e1
    nc.scalar.activation(e1[:, HN:], at_of[1][:, HN:], AF.Exp)
    mm(1, 0, KC)

    # --- tiles 2..7: singles ----------------------------------------------
    for j in range(2, 8):
        if j == 2:
            post_single(4)
        if j in (2, 3, 4):
            post_single(j + 3)
        if j == 5:
            post_single(8)
            post_single(9)
        if j == 6:
            post_single(10)
            post_single(11)
        if j == 7:
            post_single(12)
            post_single(13)
        if j == 2:
            warm_mm(at_of[2])
        exp_single(j)
        mm(j, 0, KC)
        if j >= 2:
            norm(j - 2)

    # --- tiles 8..13: singles ---------------------------------------------
    for j in range(8, 14):
        if j == 8:
            post_single(14)
            post_single(15)
        exp_single(j)
        mm(j, 0, KC)
        norm(j - 2)

    # --- tail: tile 14 single, tile 15 as a half then two quarters so the
    # final serial exp+matmul piece in the drain chain is as small as
    # possible -------------------------------------------------------------
    QN = N // 4
    exp_single(14)
    mm(14, 0, KC)
    norm(12)
    e15 = esing.tile([128, N], f16, tag="es", name="e15")
    nc.scalar.activation(e15[:, :HN], at_of[15][:, :HN], AF.Exp)
    e_of[15] = e15
    mm(15, 0, KC // 2)
    norm(13)
    nc.scalar.activation(e15[:, HN:3 * QN], at_of[15][:, HN:3 * QN], AF.Exp)
    mm(15, KC // 2, 3 * KC // 4)
    norm(14)
    nc.scalar.activation(e15[:, 3 * QN:], at_of[15][:, 3 * QN:], AF.Exp)
    mm(15, 3 * KC // 4, KC)
    norm(15)


def _build():
    from contextlib import ExitStack

    nc = bacc.Bacc(None, target_bir_lowering=False)
    # attm[rb, p, k*128 + r] = masked_att[head, half*2048 + rb*128 + r, k*128 + p]
    attm = nc.dram_tensor("attm", [RBLKS, 128, N], f16, kind="ExternalInput")
    # xb16[p, k*257 + j] = x[head, k*128 + p, j] (j<256), 1.0 (j=256)
    xb16 = nc.dram_tensor("xb16", [128, KC * DP1], f16, kind="ExternalInput")
    # out[p, rb, d] = result row rb*128 + p of this core's 2048-row slice
    out = nc.dram_tensor("out", [128, RBLKS, D], f16, kind="ExternalOutput")
    with tile.TileContext(nc) as tc, ExitStack() as ctx:
        _emit(ctx, tc, attm.ap(), xb16.ap(), out.ap())
    nc.compile()
    return nc


_PROGRAM = None


def _get_program():
    global _PROGRAM
    if _PROGRAM is None:
        _PROGRAM = _build()
    return _PROGRAM


def make_in_maps(x, adj, att_pattern):
    x32 = np.asarray(x, dtype=np.float32)
    att16 = np.asarray(att_pattern, dtype=np.float32).astype(np.float16)
    adjb = np.asarray(adj) != 0

    # Mask and leaky_relu folded into the score tensor on the host:
    # masked -> -60, which the device's exp turns into an exact 0.
    leaky = np.maximum(att16, att16 * np.float16(0.2))
    attm = np.where(adjb[None, :, :], leaky, MASKVAL)  # [H, N, N] f16

    # x with ones column, pre-arranged so each head is one contiguous-per-
    # partition DMA: [H, 128, KC*(D+1)] f16.
    xaug = np.empty((H, N, DP1), dtype=np.float16)
    xaug[:, :, :D] = x32.astype(np.float16)
    xaug[:, :, D] = np.float16(1.0)
    xb = np.ascontiguousarray(
        xaug.reshape(H, KC, 128, DP1).transpose(0, 2, 1, 3)
    ).reshape(H, 128, KC * DP1)

    in_maps = []
    for c in range(NCORES):
        h, half = divmod(c, 2)
        rows = attm[h, half * R:(half + 1) * R, :]         # [2048, 4096]
        # attm_t[rb, p, k*128 + r] = rows[rb*128 + r, k*128 + p]
        t = rows.reshape(RBLKS, 128, KC, 128).transpose(0, 3, 2, 1)
        in_maps.append({
            "attm": np.ascontiguousarray(t).reshape(RBLKS, 128, N),
            "xb16": xb[h],
        })
    return in_maps


def unshard(results):
    """results: per-core dicts with out [128, RBLKS, D] f16 -> [H, N, D] f32."""
    per_core = [
        np.ascontiguousarray(np.swapaxes(r["out"], 0, 1)).reshape(R, D)
        for r in results
    ]
    heads = [np.concatenate([per_core[2 * h], per_core[2 * h + 1]], axis=0)
             for h in range(H)]
    return np.stack(heads).astype(np.float32)


def kernel(x, adj, att_pattern, is_val=0, epoch=1, layer_position=0,
           **_unused):
    nc = _get_program()
    in_maps = make_in_maps(x, adj, att_pattern)
    res = run_bass_kernel_spmd(nc, in_maps, core_ids=list(range(NCORES)))
    return unshard(res.results)


# revision 34
# speedup vs baseline: 1.0073x; 1.0073x over previous
"""Trainium2 Bass kernel for masked-softmax attention (sparse_attention).

Computes, for full inputs
    x           [H=4, N=4096, D=256] f32
    adj         [N, N] int32 (0/1)
    att_pattern [H, N, N] f32
the reference
    score = leaky_relu(att_pattern, 0.2)
    score = where(adj > 0, score, -9e15)
    ratio = softmax(score, axis=-1)
    out   = einsum('hnm,hmd->hnd', ratio, x)

Sharding: head-parallel (per the sharding hint) — core c handles head c//2,
row half c%2 (2048 rows), so each core needs only its own head's x (2.1MB)
instead of a replicated 8.4MB slab.

Host-side marshalling: adj and the elementwise leaky_relu are folded into
the score tensor on the host (s = where(adj, leaky_relu(att), -60) in f16;
exp(-60) -> 0 exactly), so the mask costs zero HBM traffic and the
score-prep costs zero DVE work on device. Scores ship f16 PRE-TRANSPOSED
into the [m-on-partitions, rows-free] layout the PE matmul wants for lhsT.
x ships f16 with a ones-column appended (the ones column makes the
accumulating matmul produce row-sums for free).

The device computes the softmax-attention proper, per 128-row tile
(at = masked score^T tile, f16):
    e  = exp(at)                  (ACT; scores <= ~5.7 so e <= ~300, no
                                   max-subtraction needed; ~59us of exp)
    psum[rows, 0:256] += e.T @ x_chunk ; psum[rows, 256] += rowsum(e)
                                  (PE; 32 accumulating 128x128 @ 128x257
                                   matmuls per tile, ~62us warm — the
                                   pacing engine)
    out_rows = psum[:, :256] * (1 / psum[:, 256])   (DVE normalize, lagged
                                   two tiles so it never idles on PSUM)
fp16 data path, fp32 PSUM accumulation, f16 output (host casts f32).

DMA (~20MB at ~330GB/s paces the ramp): att streams as 1MB singles — the
first two tiles as 0.5MB halves — with the x slab interleaved as four
0.53MB quarters so the first matmuls start at ~13us without displacing the
att deliveries the exp stream needs; an 8-deep att ring keeps DMA posts
from ever WAR-waiting on exp completions. Output leaves in three stores
(8/7/1 tiles) so the drain chain ends on a 0.13MB transfer. Tiny warm-up
matmuls tied to the first att arrivals keep the HAM clock gate from
re-throttling PE across ramp gaps.
"""

import numpy as np

import concourse.bass as bass
import concourse.mybir as mybir
import concourse.tile as tile
from concourse import bacc
from concourse.bass_utils import run_bass_kernel_spmd

H, N, D = 4, 4096, 256
NCORES = 8
R = N // 2               # rows per core = 2048 (half a head)
RBLKS = R // 128         # 128-row blocks per core = 16
KC = N // 128            # contraction chunks = 32
DP1 = D + 1              # matmul rhs width (ones column appended)
HN = N // 2              # half a tile's free dim (= chunks 0..15)
MASKVAL = np.float16(-60.0)   # exp(-60) -> 0 exactly in f16

f32 = mybir.dt.float32
f16 = mybir.dt.float16
AF = mybir.ActivationFunctionType
OP = mybir.AluOpType

def _emit(ctx, tc: tile.TileContext, attm: bass.AP, xb16: bass.AP,
          out: bass.AP):
    nc = tc.nc

    xpool = ctx.enter_context(tc.tile_pool(name="xpool", bufs=1))
    # 8-deep singles ring: the att stream must never wait on exp completions
    # (a 4-deep ring made every DMA post WAR-wait on an exp 4 tiles back).
    atsing = ctx.enter_context(tc.tile_pool(name="atsing", bufs=10))
    esing = ctx.enter_context(tc.tile_pool(name="esing", bufs=5))
    opool = ctx.enter_context(tc.tile_pool(name="opool", bufs=3))
    rpool = ctx.enter_context(tc.tile_pool(name="rpool", bufs=2))
    psum_o = ctx.enter_context(tc.tile_pool(name="psum_o", bufs=6, space="PSUM"))

    # x slab for this core's head, loaded once as four 0.53MB quarters
    # interleaved between the first att tiles: tile 0's matmuls start after
    # the first quarter, and the 3.7us dribble keeps PE busy often enough
    # that the HAM clock-gate never re-throttles it during the ramp.
    xs = xpool.tile([128, KC, DP1], f16, tag="xs", name="xs")
    KQ = KC // 4

    def post_xs_quarter(q):
        nc.sync.dma_start(
            xs[:, q * KQ:(q + 1) * KQ, :],
            xb16[:, q * KQ * DP1:(q + 1) * KQ * DP1]
            .rearrange("p (k d) -> p k d", k=KQ))

    at_of = {}               # tile index -> its [128, N] AP

    def post_single(j, parts=1):
        at = atsing.tile([128, N], f16, tag="ats", name=f"at{j}")
        at_of[j] = at
        step = N // parts
        for p in range(parts):
            nc.sync.dma_start(at[:, p * step:(p + 1) * step],
                              attm[j][:, p * step:(p + 1) * step])

    # att delivery: tiles 0,1 in 0.5MB halves (earliest possible exp starts)
    # with xs quarters interleaved, then 1MB singles throughout — pair-sized
    # transfers made the exp stream wait on 2MB-atomic deliveries.
    post_single(0, parts=4)
    post_xs_quarter(0)
    post_single(1, parts=2)
    post_xs_quarter(1)
    post_single(2)
    post_xs_quarter(2)
    post_single(3)
    post_xs_quarter(3)

    e_of = {}
    po_of = {}
    obufs = {}

    def mm(j, ks, ke):
        """accumulate psum[j] over contraction chunks [ks, ke)."""
        if j not in po_of:
            po_of[j] = psum_o.tile([128, DP1], f32, tag="po", name=f"po{j}")
        po = po_of[j]
        e = e_of[j]
        for kk in range(ks, ke):
            nc.tensor.matmul(
                po,
                lhsT=e[:, kk * 128:(kk + 1) * 128],
                rhs=xs[:, kk, :],
                start=(kk == 0),
                stop=(kk == KC - 1),
            )

    # PE clock-gate warm-up: the HAM re-throttles PE to 1.2GHz after ~5.2us
    # idle, so 65ns matmuls tied to the first att arrivals bridge the ramp
    # gaps between PE's preamble and the real matmul stream. The scratch
    # PSUM is never read.
    wpool = ctx.enter_context(tc.tile_pool(name="wpool", bufs=1))
    wscr = wpool.tile([128, 8], f16, tag="wscr", name="wscr")
    nc.gpsimd.memset(wscr, 0.0)
    psum_w = ctx.enter_context(tc.tile_pool(name="psum_w", bufs=1, space="PSUM"))
    warm_psum = psum_w.tile([128, 8], f32, tag="warm", name="warm_psum")

    def warm_mm(src):
        nc.tensor.matmul(warm_psum, lhsT=src[:, :128], rhs=wscr,
                         start=True, stop=True)

    # output store groups: tiles 0-7, 8-14, then 15 alone so the final
    # store in the drain chain is only 0.13MB
    OGRP = {j: (0, j, 8) for j in range(8)}
    OGRP.update({j: (1, j - 8, 7) for j in range(8, 15)})
    OGRP[15] = (2, 0, 1)
    OBASE = {0: 0, 1: 8, 2: 15}

    def norm(j):
        po = po_of[j]
        rec = rpool.tile([128, 1], f32, tag="rec", name=f"rec{j}")
        nc.vector.reciprocal(rec, po[:, D:DP1])
        g, slot, gsize = OGRP[j]
        if slot == 0:
            obufs[g] = opool.tile([128, 8, D], f16, tag="o", name=f"o{g}")
        nc.vector.tensor_scalar_mul(obufs[g][:, slot, :], po[:, :D], rec)
        if slot == gsize - 1:
            nc.sync.dma_start(out[:, OBASE[g]:OBASE[g] + gsize, :],
                              obufs[g][:, :gsize, :])

    def exp_single(j):
        e = esing.tile([128, N], f16, tag="es", name=f"e{j}")
        nc.scalar.activation(e, at_of[j], AF.Exp)
        e_of[j] = e

    # --- tile 0: quarter-granular exp and matmuls (0.25MB first delivery
    # lets PE start ~1.5us earlier inside the fixed cold-clock window) -----
    QN0 = N // 4
    warm_mm(at_of[0][:, :QN0])
    e0 = esing.tile([128, N], f16, tag="es", name="e0")
    for q in range(4):
        nc.scalar.activation(e0[:, q * QN0:(q + 1) * QN0],
                             at_of[0][:, q * QN0:(q + 1) * QN0], AF.Exp)
        if q == 0:
            warm_mm(at_of[0][:, QN0:2 * QN0])
        e_of[0] = e0
        mm(0, q * KQ, (q + 1) * KQ)
    e1 = esing.tile([128, N], f16, tag="es", name="e1")
    nc.scalar.activation(e1[:, :HN], at_of[1][:, :HN], AF.Exp)
    e_of[1] = e1
    nc.scalar.activation(e1[:, HN:], at_of[1][:, HN:], AF.Exp)
    mm(1, 0, KC)

    # --- tiles 2..7: singles ----------------------------------------------
    for j in range(2, 8):
        if j == 2:
            post_single(4)
        if j in (2, 3, 4):
            post_single(j + 3)
        if j == 5:
            post_single(8)
            post_single(9)
        if j == 6:
            post_single(10)
            post_single(11)
        if j == 7:
            post_single(12)
            post_single(13)
        if j == 2:
            warm_mm(at_of[2])
        exp_single(j)
        mm(j, 0, KC)
        if j >= 2:
            norm(j - 2)

    # --- tiles 8..13: singles ---------------------------------------------
    for j in range(8, 14):
        if j == 8:
            post_single(14)
            post_single(15)
        exp_single(j)
        mm(j, 0, KC)
        norm(j - 2)

    # --- tail: tile 14 single, tile 15 as a half then two quarters so the
    # final serial exp+matmul piece in the drain chain is as small as
    # possible -------------------------------------------------------------
    QN = N // 4
    exp_single(14)
    mm(14, 0, KC)
    norm(12)
    e15 = esing.tile([128, N], f16, tag="es", name="e15")
    nc.scalar.activation(e15[:, :HN], at_of[15][:, :HN], AF.Exp)
    e_of[15] = e15
    mm(15, 0, KC // 2)
    norm(13)
    nc.scalar.activation(e15[:, HN:3 * QN], at_of[15][:, HN:3 * QN], AF.Exp)
    mm(15, KC // 2, 3 * KC // 4)
    norm(14)
    nc.scalar.activation(e15[:, 3 * QN:], at_of[15][:, 3 * QN:], AF.Exp)
    mm(15, 3 * KC // 4, KC)
    norm(15)


def _build():
    from contextlib import ExitStack

    nc = bacc.Bacc(None, target_bir_lowering=False)
    # attm[rb, p, k*128 + r] = masked_att[head, half*2048 + rb*128 + r, k*128 + p]
    attm = nc.dram_tensor("attm", [RBLKS, 128, N], f16, kind="ExternalInput")
    # xb16[p, k*257 + j] = x[head, k*128 + p, j] (j<256), 1.0 (j=256)
    xb16 = nc.dram_tensor("xb16", [128, KC * DP1], f16, kind="ExternalInput")
    # out[p, rb, d] = result row rb*128 + p of this core's 2048-row slice
    out = nc.dram_tensor("out", [128, RBLKS, D], f16, kind="ExternalOutput")
    with tile.TileContext(nc) as tc, ExitStack() as ctx:
        _emit(ctx, tc, attm.ap(), xb16.ap(), out.ap())
    nc.compile()
    return nc


_PROGRAM = None


def _get_program():
    global _PROGRAM
    if _PROGRAM is None:
        _PROGRAM = _build()
    return _PROGRAM


def make_in_maps(x, adj, att_pattern):
    x32 = np.asarray(x, dtype=np.float32)
    att16 = np.asarray(att_pattern, dtype=np.float32).astype(np.float16)
    adjb = np.asarray(adj) != 0

    # Mask and leaky_relu folded into the score tensor on the host:
    # masked -> -60, which the device's exp turns into an exact 0.
    leaky = np.maximum(att16, att16 * np.float16(0.2))
    attm = np.where(adjb[None, :, :], leaky, MASKVAL)  # [H, N, N] f16

    # x with ones column, pre-arranged so each head is one contiguous-per-
    # partition DMA: [H, 128, KC*(D+1)] f16.
    xaug = np.empty((H, N, DP1), dtype=np.float16)
    xaug[:, :, :D] = x32.astype(np.float16)
    xaug[:, :, D] = np.float16(1.0)
    xb = np.ascontiguousarray(
        xaug.reshape(H, KC, 128, DP1).transpose(0, 2, 1, 3)
    ).reshape(H, 128, KC * DP1)

    in_maps = []
    for c in range(NCORES):
        h, half = divmod(c, 2)
        rows = attm[h, half * R:(half + 1) * R, :]         # [2048, 4096]
        # attm_t[rb, p, k*128 + r] = rows[rb*128 + r, k*128 + p]
        t = rows.reshape(RBLKS, 128, KC, 128).transpose(0, 3, 2, 1)
        in_maps.append({
            "attm": np.ascontiguousarray(t).reshape(RBLKS, 128, N),
            "xb16": xb[h],
        })
    return in_maps


def unshard(results):
    """results: per-core dicts with out [128, RBLKS, D] f16 -> [H, N, D] f32."""
    per_core = [
        np.ascontiguousarray(np.swapaxes(r["out"], 0, 1)).reshape(R, D)
        for r in results
    ]
    heads = [np.concatenate([per_core[2 * h], per_core[2 * h + 1]], axis=0)
             for h in range(H)]
    return np.stack(heads).astype(np.float32)


def kernel(x, adj, att_pattern, is_val=0, epoch=1, layer_position=0,
           **_unused):
    nc = _get_program()
    in_maps = make_in_maps(x, adj, att_pattern)
    res = run_bass_kernel_spmd(nc, in_maps, core_ids=list(range(NCORES)))
    return unshard(res.results)


# revision 35
# speedup vs baseline: 1.0181x; 1.0107x over previous
"""Trainium2 Bass kernel for masked-softmax attention (sparse_attention).

Computes, for full inputs
    x           [H=4, N=4096, D=256] f32
    adj         [N, N] int32 (0/1)
    att_pattern [H, N, N] f32
the reference
    score = leaky_relu(att_pattern, 0.2)
    score = where(adj > 0, score, -9e15)
    ratio = softmax(score, axis=-1)
    out   = einsum('hnm,hmd->hnd', ratio, x)

Sharding: head-parallel (per the sharding hint) — core c handles head c//2,
row half c%2 (2048 rows), so each core needs only its own head's x (2.1MB)
instead of a replicated 8.4MB slab.

Host-side marshalling: adj and the elementwise leaky_relu are folded into
the score tensor on the host (s = where(adj, leaky_relu(att), -60) in f16;
exp(-60) -> 0 exactly), so the mask costs zero HBM traffic and the
score-prep costs zero DVE work on device. Scores ship f16 PRE-TRANSPOSED
into the [m-on-partitions, rows-free] layout the PE matmul wants for lhsT.
x ships f16 with a ones-column appended (the ones column makes the
accumulating matmul produce row-sums for free).

The device computes the softmax-attention proper, per 128-row tile
(at = masked score^T tile, f16):
    e  = exp(at)                  (ACT; scores <= ~5.7 so e <= ~300, no
                                   max-subtraction needed; ~59us of exp)
    psum[rows, 0:256] += e.T @ x_chunk ; psum[rows, 256] += rowsum(e)
                                  (PE; 32 accumulating 128x128 @ 128x257
                                   matmuls per tile, ~62us warm — the
                                   pacing engine)
    out_rows = psum[:, :256] * (1 / psum[:, 256])   (DVE normalize, lagged
                                   two tiles so it never idles on PSUM)
fp16 data path, fp32 PSUM accumulation, f16 output (host casts f32).

DMA (~20MB at ~330GB/s paces the ramp): att streams as 1MB singles — the
first two tiles as 0.5MB halves — with the x slab interleaved as four
0.53MB quarters so the first matmuls start at ~13us without displacing the
att deliveries the exp stream needs; an 8-deep att ring keeps DMA posts
from ever WAR-waiting on exp completions. Output leaves in three stores
(8/7/1 tiles) so the drain chain ends on a 0.13MB transfer. Tiny warm-up
matmuls tied to the first att arrivals keep the HAM clock gate from
re-throttling PE across ramp gaps.
"""

import numpy as np

import concourse.bass as bass
import concourse.mybir as mybir
import concourse.tile as tile
from concourse import bacc
from concourse.bass_utils import run_bass_kernel_spmd

H, N, D = 4, 4096, 256
NCORES = 8
R = N // 2               # rows per core = 2048 (half a head)
RBLKS = R // 128         # 128-row blocks per core = 16
KC = N // 128            # contraction chunks = 32
DP1 = D + 1              # matmul rhs width (ones column appended)
HN = N // 2              # half a tile's free dim (= chunks 0..15)
MASKVAL = np.float16(-60.0)   # exp(-60) -> 0 exactly in f16

f32 = mybir.dt.float32
f16 = mybir.dt.float16
AF = mybir.ActivationFunctionType
OP = mybir.AluOpType

def _emit(ctx, tc: tile.TileContext, attm: bass.AP, xb16: bass.AP,
          out: bass.AP):
    nc = tc.nc

    xpool = ctx.enter_context(tc.tile_pool(name="xpool", bufs=1))
    # 8-deep singles ring: the att stream must never wait on exp completions
    # (a 4-deep ring made every DMA post WAR-wait on an exp 4 tiles back).
    atsing = ctx.enter_context(tc.tile_pool(name="atsing", bufs=10))
    esing = ctx.enter_context(tc.tile_pool(name="esing", bufs=5))
    opool = ctx.enter_context(tc.tile_pool(name="opool", bufs=3))
    rpool = ctx.enter_context(tc.tile_pool(name="rpool", bufs=2))
    psum_o = ctx.enter_context(tc.tile_pool(name="psum_o", bufs=6, space="PSUM"))

    # x slab for this core's head, loaded once as four 0.53MB quarters
    # interleaved between the first att tiles: tile 0's matmuls start after
    # the first quarter, and the 3.7us dribble keeps PE busy often enough
    # that the HAM clock-gate never re-throttles it during the ramp.
    xs = xpool.tile([128, KC, DP1], f16, tag="xs", name="xs")
    KQ = KC // 4

    def post_xs_quarter(q):
        nc.sync.dma_start(
            xs[:, q * KQ:(q + 1) * KQ, :],
            xb16[:, q * KQ * DP1:(q + 1) * KQ * DP1]
            .rearrange("p (k d) -> p k d", k=KQ))

    at_of = {}               # tile index -> its [128, N] AP

    def post_single(j, parts=1):
        at = atsing.tile([128, N], f16, tag="ats", name=f"at{j}")
        at_of[j] = at
        step = N // parts
        for p in range(parts):
            nc.sync.dma_start(at[:, p * step:(p + 1) * step],
                              attm[j][:, p * step:(p + 1) * step])

    # att delivery: tiles 0,1 in 0.5MB halves (earliest possible exp starts)
    # with xs quarters interleaved, then 1MB singles throughout — pair-sized
    # transfers made the exp stream wait on 2MB-atomic deliveries.
    post_single(0, parts=2)
    post_xs_quarter(0)
    post_single(1, parts=2)
    post_xs_quarter(1)
    post_single(2)
    post_xs_quarter(2)
    post_single(3)
    post_xs_quarter(3)

    e_of = {}
    po_of = {}
    obufs = {}

    def mm(j, ks, ke):
        """accumulate psum[j] over contraction chunks [ks, ke)."""
        if j not in po_of:
            po_of[j] = psum_o.tile([128, DP1], f32, tag="po", name=f"po{j}")
        po = po_of[j]
        e = e_of[j]
        for kk in range(ks, ke):
            nc.tensor.matmul(
                po,
                lhsT=e[:, kk * 128:(kk + 1) * 128],
                rhs=xs[:, kk, :],
                start=(kk == 0),
                stop=(kk == KC - 1),
            )

    # PE clock-gate warm-up: the HAM re-throttles PE to 1.2GHz after ~5.2us
    # idle, so 65ns matmuls tied to the first att arrivals bridge the ramp
    # gaps between PE's preamble and the real matmul stream. The scratch
    # PSUM is never read.
    wpool = ctx.enter_context(tc.tile_pool(name="wpool", bufs=1))
    wscr = wpool.tile([128, 8], f16, tag="wscr", name="wscr")
    nc.gpsimd.memset(wscr, 0.0)
    psum_w = ctx.enter_context(tc.tile_pool(name="psum_w", bufs=1, space="PSUM"))
    warm_psum = psum_w.tile([128, 8], f32, tag="warm", name="warm_psum")

    def warm_mm(src):
        nc.tensor.matmul(warm_psum, lhsT=src[:, :128], rhs=wscr,
                         start=True, stop=True)

    # output store groups: tiles 0-7, 8-14, then 15 alone so the final
    # store in the drain chain is only 0.13MB
    OGRP = {j: (0, j, 8) for j in range(8)}
    OGRP.update({j: (1, j - 8, 7) for j in range(8, 15)})
    OGRP[15] = (2, 0, 1)
    OBASE = {0: 0, 1: 8, 2: 15}

    def norm(j):
        po = po_of[j]
        rec = rpool.tile([128, 1], f32, tag="rec", name=f"rec{j}")
        nc.vector.reciprocal(rec, po[:, D:DP1])
        g, slot, gsize = OGRP[j]
        if slot == 0:
            obufs[g] = opool.tile([128, 8, D], f16, tag="o", name=f"o{g}")
        nc.vector.tensor_scalar_mul(obufs[g][:, slot, :], po[:, :D], rec)
        if slot == gsize - 1:
            nc.sync.dma_start(out[:, OBASE[g]:OBASE[g] + gsize, :],
                              obufs[g][:, :gsize, :])

    def exp_single(j):
        e = esing.tile([128, N], f16, tag="es", name=f"e{j}")
        nc.scalar.activation(e, at_of[j], AF.Exp)
        e_of[j] = e

    # --- tiles 0,1: exp in halves, tile-0 matmuls split per xs quarter ----
    warm_mm(at_of[0][:, :HN])
    e0 = esing.tile([128, N], f16, tag="es", name="e0")
    nc.scalar.activation(e0[:, :HN], at_of[0][:, :HN], AF.Exp)
    e_of[0] = e0
    warm_mm(at_of[0][:, HN:])
    mm(0, 0, KQ)
    mm(0, KQ, 2 * KQ)
    nc.scalar.activation(e0[:, HN:], at_of[0][:, HN:], AF.Exp)
    mm(0, 2 * KQ, 3 * KQ)
    mm(0, 3 * KQ, KC)
    e1 = esing.tile([128, N], f16, tag="es", name="e1")
    nc.scalar.activation(e1[:, :HN], at_of[1][:, :HN], AF.Exp)
    e_of[1] = e1
    nc.scalar.activation(e1[:, HN:], at_of[1][:, HN:], AF.Exp)
    mm(1, 0, KC)

    # --- tiles 2..7: singles ----------------------------------------------
    for j in range(2, 8):
        if j == 2:
            post_single(4)
        if j in (2, 3, 4):
            post_single(j + 3)
        if j == 5:
            post_single(8)
            post_single(9)
        if j == 6:
            post_single(10)
            post_single(11)
        if j == 7:
            post_single(12)
            post_single(13)
        if j == 2:
            warm_mm(at_of[2])
        exp_single(j)
        mm(j, 0, KC)
        if j >= 2:
            norm(j - 2)

    # --- tiles 8..13: singles ---------------------------------------------
    for j in range(8, 14):
        if j == 8:
            post_single(14)
            post_single(15)
        exp_single(j)
        mm(j, 0, KC)
        norm(j - 2)

    # --- tail: tile 14 single, tile 15 as a half then two quarters so the
    # final serial exp+matmul piece in the drain chain is as small as
    # possible -------------------------------------------------------------
    QN = N // 4
    exp_single(14)
    mm(14, 0, KC)
    norm(12)
    e15 = esing.tile([128, N], f16, tag="es", name="e15")
    nc.scalar.activation(e15[:, :HN], at_of[15][:, :HN], AF.Exp)
    e_of[15] = e15
    mm(15, 0, KC // 2)
    norm(13)
    nc.scalar.activation(e15[:, HN:3 * QN], at_of[15][:, HN:3 * QN], AF.Exp)
    mm(15, KC // 2, 3 * KC // 4)
    norm(14)
    nc.scalar.activation(e15[:, 3 * QN:], at_of[15][:, 3 * QN:], AF.Exp)
    mm(15, 3 * KC // 4, KC)
    norm(15)


def _build():
    from contextlib import ExitStack

    nc = bacc.Bacc(None, target_bir_lowering=False)
    # attm[rb, p, k*128 + r] = masked_att[head, half*2048 + rb*128 + r, k*128 + p]
    attm = nc.dram_tensor("attm", [RBLKS, 128, N], f16, kind="ExternalInput")
    # xb16[p, k*257 + j] = x[head, k*128 + p, j] (j<256), 1.0 (j=256)
    xb16 = nc.dram_tensor("xb16", [128, KC * DP1], f16, kind="ExternalInput")
    # out[p, rb, d] = result row rb*128 + p of this core's 2048-row slice
    out = nc.dram_tensor("out", [128, RBLKS, D], f16, kind="ExternalOutput")
    with tile.TileContext(nc) as tc, ExitStack() as ctx:
        _emit(ctx, tc, attm.ap(), xb16.ap(), out.ap())
    nc.compile()
    return nc


_PROGRAM = None


def _get_program():
    global _PROGRAM
    if _PROGRAM is None:
        _PROGRAM = _build()
    return _PROGRAM


def make_in_maps(x, adj, att_pattern):
    x32 = np.asarray(x, dtype=np.float32)
    att16 = np.asarray(att_pattern, dtype=np.float32).astype(np.float16)
    adjb = np.asarray(adj) != 0

    # Mask and leaky_relu folded into the score tensor on the host:
    # masked -> -60, which the device's exp turns into an exact 0.
    leaky = np.maximum(att16, att16 * np.float16(0.2))
    attm = np.where(adjb[None, :, :], leaky, MASKVAL)  # [H, N, N] f16

    # x with ones column, pre-arranged so each head is one contiguous-per-
    # partition DMA: [H, 128, KC*(D+1)] f16.
    xaug = np.empty((H, N, DP1), dtype=np.float16)
    xaug[:, :, :D] = x32.astype(np.float16)
    xaug[:, :, D] = np.float16(1.0)
    xb = np.ascontiguousarray(
        xaug.reshape(H, KC, 128, DP1).transpose(0, 2, 1, 3)
    ).reshape(H, 128, KC * DP1)

    in_maps = []
    for c in range(NCORES):
        h, half = divmod(c, 2)
        rows = attm[h, half * R:(half + 1) * R, :]         # [2048, 4096]
        # attm_t[rb, p, k*128 + r] = rows[rb*128 + r, k*128 + p]
        t = rows.reshape(RBLKS, 128, KC, 128).transpose(0, 3, 2, 1)
        in_maps.append({
            "attm": np.ascontiguousarray(t).reshape(RBLKS, 128, N),
            "xb16": xb[h],
        })
    return in_maps


def unshard(results):
    """results: per-core dicts with out [128, RBLKS, D] f16 -> [H, N, D] f32."""
    per_core = [
        np.ascontiguousarray(np.swapaxes(r["out"], 0, 1)).reshape(R, D)
        for r in results
    ]
    heads = [np.concatenate([per_core[2 * h], per_core[2 * h + 1]], axis=0)
             for h in range(H)]
    return np.stack(heads).astype(np.float32)


def kernel(x, adj, att_pattern, is_val=0, epoch=1, layer_position=0,
           **_unused):
    nc = _get_program()
    in_maps = make_in_maps(x, adj, att_pattern)
    res = run_bass_kernel_spmd(nc, in_maps, core_ids=list(range(NCORES)))
    return unshard(res.results)
